# revision 32
# baseline (speedup 1.0000x reference)
"""GQA attention (B=2,S=2048,DIM=4096,NH=32,NKV=8,HD=128) on 8 TRN2 NeuronCores.

Tensor-parallel over KV groups: core c owns q-heads [4c,4c+4), kv-head c and
wo columns [512c,512c+512). x replicated (feature-major fp16); each core emits
a partial (T,DIM) fp16 wo-output; host sums the 8 partials in fp32.

Fused-pipeline design (~788us vs 1012us for the 3-phase serial baseline):
- One instruction stream, overlapped windows: attention(b0) interleaves
  (pull-based, ~2 steps per 4 projection matmuls) into batch-1 QKV GEMMs,
  attention(b1) into batch-0 wo GEMMs.  The scalar-engine-bound softmax
  (~26us/head of Exp) hides behind peak-rate PE streams, and the PE runs at
  its 216ns/512-col issue rate through phase transitions.
- q/k/v and attention outputs live entirely in SBUF (no DRAM roundtrips).
- RoPE pair-swap via two contiguous partition-block DMA copies (wq/wk rows
  de-interleaved per head on host); V transpose on the DMA xbar engine;
  zero PE/PSUM cost for either.
- Softmax denominators: per 4 score tiles, 4 column-strip matmuls
  (tile_position=(0,32g)) issued back-to-back execute concurrently
  (~310ns per quad instead of 4 full streams), accumulated in one PSUM
  bank; strips summed by a [128,4] select-matmul, 1/den via
  reciprocal_approx_fast, broadcast over partitions by a bf16 rank-1
  matmul.
- All DRAM operands pre-tiled on host so every load is a contiguous
  >=8KB-per-partition burst; x streams as 1MB quarters over the three DMA
  issue queues (sync/scalar/gpsimd), double-buffered one ti ahead.
- P1 runs in 3-slice waves (kg-outer) matching first-chunk DMA arrival.
- PSUM: P1 3 accs + 3 score + 1 attn-out + 1 den (windows C/D retire the
  P1/P2 pools for 3/6 wo-accumulator banks).
"""

import math

import ml_dtypes
import numpy as np

import concourse.bass as bass
import concourse.mybir as mybir
import concourse.tile as tile
from concourse import bacc
from concourse.bass_utils import run_bass_kernel_spmd

B, S, DIM = 2, 2048, 4096
NH, NKV, HD = 32, 8, 128
T = B * S
N_CORES = 8
QPC = (NH // N_CORES) * HD          # 512 q-dims per core
NHEAD = NH // N_CORES               # 4 q heads per core
P = 128
F32 = mybir.dt.float32
F16 = mybir.dt.float16
BF16 = mybir.dt.bfloat16
SCALE = 1.0 / math.sqrt(HD)

TT1 = 512                           # phase-1 token tile
NT1 = T // TT1                      # 8
KO = DIM // P                       # 32 contraction tiles
TT2 = 512                           # phase-2 t tile
NT2 = S // TT2                      # 4 t-tiles per (b,h)
TT3 = 128                           # phase-3 token tile
IT3 = 512                           # phase-3 output-column tile


def build_kernel() -> bass.Bass:
    nc = bacc.Bacc()

    # all big operands arrive pre-tiled so every DMA is a contiguous
    # per-partition burst (8KB+) instead of 1KB strided descriptors
    xR = nc.declare_dram_parameter("xR", [NT1, 4, P, 8, TT1], F16,
                                   isOutput=False)
    wqR = nc.declare_dram_parameter("wqR", [4, P, 8, QPC], F16,
                                    isOutput=False)
    wkR = nc.declare_dram_parameter("wkR", [P, KO, HD], F16, isOutput=False)
    wvR = nc.declare_dram_parameter("wvR", [P, KO, HD], F16, isOutput=False)
    woR = nc.declare_dram_parameter("woR", [P, QPC // P, DIM], F16,
                                    isOutput=False)
    ropeC = nc.declare_dram_parameter("ropeC", [P, S], F16, isOutput=False)
    ropeS = nc.declare_dram_parameter("ropeS", [P, S], F16, isOutput=False)
    colC4M = nc.declare_dram_parameter("colC4M", [P, 4 * NT2], BF16,
                                       isOutput=False)
    sel4M = nc.declare_dram_parameter("sel4M", [P, 4], BF16, isOutput=False)
    triM = nc.declare_dram_parameter("triM", [P, P], F32, isOutput=False)
    biasM = nc.declare_dram_parameter("biasM", [P, 1], F32, isOutput=False)
    out = nc.declare_dram_parameter("out_part", [T, DIM], F16, isOutput=True)

    with tile.TileContext(nc) as tc:
        # ------------- pools; stack allocator => LIFO release discipline.
        cpool = tc.alloc_tile_pool(name="ct", bufs=1)
        kvpool = tc.alloc_tile_pool(name="qkv", bufs=1)
        aopool0 = tc.alloc_tile_pool(name="aop0", bufs=1)
        epool = tc.alloc_tile_pool(name="ep", bufs=7)
        opool = tc.alloc_tile_pool(name="op", bufs=2)
        ps_sc = tc.alloc_tile_pool(name="p2sc", bufs=3, space="PSUM")
        ps_o = tc.alloc_tile_pool(name="p2o", bufs=1, space="PSUM")
        ps_den = tc.alloc_tile_pool(name="p2d", bufs=1, space="PSUM")
        wpool = tc.alloc_tile_pool(name="w1", bufs=1)
        xpool = tc.alloc_tile_pool(name="xp", bufs=6)
        spool = tc.alloc_tile_pool(name="sp", bufs=2)
        p1acc = tc.alloc_tile_pool(name="p1a", bufs=3, space="PSUM")

        # ------------- persistent SBUF tensors
        qT = {b: kvpool.tile([P, NHEAD, S], F16, name=f"qTb{b}")
              for b in range(B)}
        kT = {b: kvpool.tile([P, S], F16, name=f"kTb{b}") for b in range(B)}
        vS = {b: kvpool.tile([P, S // P, P], BF16, name=f"vSb{b}")
              for b in range(B)}
        aoT = {0: aopool0.tile([P, NHEAD, S], F16, name="aoTb0")}
        wo_holder = {}

        # ------------- x streaming (4 quarters per ti, multi-queue)
        xmap = {}

        def ensure_x(ti):
            if ti >= NT1 or ti in xmap:
                return
            qs = []
            if ti == 0:
                # halves land ~2x sooner; subtile deps let ko0-3 start on h0
                half_eng = [(nc.scalar, nc.scalar), (nc.sync, nc.sync),
                            (nc.scalar, nc.gpsimd), (nc.sync, nc.gpsimd)]
                for g in range(4):
                    xq = xpool.tile([P, 8, TT1], F16, tag="xq", name="xq")
                    e0, e1 = half_eng[g]
                    e0.dma_start(xq[:, 0:4, :], xR[ti, g, :, 0:4, :])
                    e1.dma_start(xq[:, 4:8, :], xR[ti, g, :, 4:8, :])
                    qs.append(xq)
                xmap[ti] = qs
                return
            for g in range(4):
                if ti < 4:
                    eng = nc.scalar if g % 2 == 0 else nc.sync
                else:
                    eng = nc.sync
                xq = xpool.tile([P, 8, TT1], F16, tag="xq", name="xq")
                eng.dma_start(xq[:], xR[ti, g])
                qs.append(xq)
            xmap[ti] = qs

        # ------------- HAM warmup: the DMA path delivers nothing for the
        # first ~10us; keep the PE busy on a zeroed scratch tile so the
        # clock-gate opens (1.2->2.4GHz) before real operands land.
        junk = cpool.tile([P, TT1], F16)
        nc.vector.memset(junk[:], 0.0)
        jps = p1acc.tile([P, TT1], F32, tag="acc", name="jps")
        for _ in range(14):
            nc.tensor.matmul(jps[:], junk[:, 0:P], junk[:],
                             start=True, stop=True)

        # ------------- weight / table loads, interleaved with ti0's x so
        # every queue streams what the first sweeps need, in order.
        wq_sb = wpool.tile([P, KO, QPC], F16)
        nc.sync.dma_start(wq_sb[:, 0:4, :], wqR[0][:, 0:4, :])
        nc.sync.dma_start(wq_sb[:, 4:8, :], wqR[0][:, 4:8, :])
        nc.gpsimd.dma_start(wq_sb[:, 8:16, :], wqR[1])
        ensure_x(0)
        nc.scalar.dma_start(wq_sb[:, 16:24, :], wqR[2])
        nc.gpsimd.dma_start(wq_sb[:, 24:32, :], wqR[3])
        wk_sb = wpool.tile([P, KO, HD], F16)
        nc.gpsimd.dma_start(wk_sb[:], wkR[:])
        wv_sb = wpool.tile([P, KO, HD], F16)
        nc.gpsimd.dma_start(wv_sb[:], wvR[:])
        ropeC_sb = wpool.tile([P, S], F16)
        nc.sync.dma_start(ropeC_sb[:], ropeC[:])
        ropeS_sb = wpool.tile([P, S], F16)
        nc.sync.dma_start(ropeS_sb[:], ropeS[:])
        colC4_sb = cpool.tile([P, 4 * NT2], BF16)
        nc.sync.dma_start(colC4_sb[:], colC4M[:])
        sel4_sb = cpool.tile([P, 4], BF16)
        nc.sync.dma_start(sel4_sb[:], sel4M[:])
        tri_sb = cpool.tile([P, P], F32)
        nc.sync.dma_start(tri_sb[:], triM[:])
        bias_sb = cpool.tile([P, 1], F32)
        nc.sync.dma_start(bias_sb[:], biasM[:])
        ones32 = cpool.tile([1, P], BF16)
        nc.gpsimd.memset(ones32[:], 1.0)

        # ================= Phase 1 machinery ==============================
        def rope_tail(b, lt0, j, acc):
            def go():
                raw = spool.tile([P, TT1], F16, tag="raw", name="raw")
                nc.any.tensor_copy(raw[:], acc[:])
                # pair-swap == swap of the (deinterleaved) top/bottom halves
                swp = spool.tile([P, TT1], F16, tag="swp", name="swp")
                nc.gpsimd.dma_start(swp[0:64, :], raw[64:128, :])
                nc.gpsimd.dma_start(swp[64:128, :], raw[0:64, :])
                rc = spool.tile([P, TT1], F16, tag="rc", name="rc")
                nc.vector.tensor_mul(rc[:], raw[:], ropeC_sb[:, lt0:lt0 + TT1])
                rs = spool.tile([P, TT1], F16, tag="rs", name="rs")
                nc.vector.tensor_mul(rs[:], swp[:], ropeS_sb[:, lt0:lt0 + TT1])
                dst = (qT[b][:, j, lt0:lt0 + TT1] if j < 4
                       else kT[b][:, lt0:lt0 + TT1])
                nc.vector.tensor_add(dst, rc[:], rs[:])
            return go

        def v_tail(b, lt0, acc):
            def go():
                vraw = spool.tile([P, TT1], BF16, tag="vraw", name="vraw")
                nc.any.tensor_copy(vraw[:], acc[:])
                so0 = lt0 // P
                for jj in range(TT1 // P):
                    nc.sync.dma_start_transpose(
                        vS[b][:, so0 + jj, :], vraw[:, jj * P:(jj + 1) * P])
            return go

        def w_of(a):
            if a < 4:
                return wq_sb, a * P
            return (wk_sb, 0) if a == 4 else (wv_sb, 0)

        p1pend = []

        def p1_ti(ti, pull):
            b = ti // (NT1 // B)
            lt0 = (ti % (NT1 // B)) * TT1
            ensure_x(ti)
            qs = xmap[ti]
            # ti0: 2-slice waves consume chunks at the DMA arrival rate
            waves = ([(0, 1), (2, 3), (4, 5)] if ti == 0
                     else [(0, 1, 2), (3, 4, 5)])
            for wi, sl in enumerate(waves):
                # previous wave's tails must retire before accs rotate
                for fn in p1pend:
                    fn()
                p1pend.clear()
                if pull:
                    pull()
                accs = [p1acc.tile([P, TT1], F32, tag="acc", name="acc")
                        for _ in sl]
                for kg in range(4):
                    for ai, a in enumerate(sl):
                        w_sb, c0 = w_of(a)
                        for k8 in range(8):
                            ko = kg * 8 + k8
                            nc.tensor.matmul(
                                accs[ai][:], w_sb[:, ko, c0:c0 + P],
                                qs[kg][:, k8, :],
                                start=(ko == 0), stop=(ko == KO - 1),
                            )
                            if pull and k8 == 3:
                                pull()
                        if pull:
                            pull()
                if wi == 0:
                    ensure_x(ti + 1)
                for ai, a in enumerate(sl):
                    if a < 5:
                        p1pend.append(rope_tail(b, lt0, a, accs[ai]))
                    else:
                        p1pend.append(v_tail(b, lt0, accs[ai]))

        # ================= Phase 2 step list ==============================
        def build_p2_steps(b):
            steps = []
            pending = []            # (stage, closure) deferred to next head

            def flush_into(stage):
                keep = []
                for stg, fn in pending:
                    if stg == stage:
                        steps.append(fn)
                    else:
                        keep.append((stg, fn))
                pending[:] = keep

            for h in range(NHEAD):
                hs = {}

                def mk_scores(t_idx, st, h=h, hs=hs, b=b):
                    def go():
                        if t_idx == 0 and st == 0:
                            hs["psden"] = ps_den.tile([P, TT2], F32,
                                                      tag="psden", name="psden")
                            hs["aouns"] = []
                        if st == 0:
                            hs["psO"] = ps_o.tile([P, TT2], F32, tag="psO",
                                                  name="psO")
                        t0 = t_idx * TT2
                        pssc = ps_sc.tile([P, TT2], F32, tag="pssc",
                                          name="pssc")
                        nc.tensor.matmul(
                            pssc[:], kT[b][:, st * P:(st + 1) * P],
                            qT[b][:, h, t0:t0 + TT2], start=True, stop=True,
                        )
                        dg = st - 4 * t_idx
                        esb = epool.tile([P, TT2], BF16, tag="esb", name="esb")
                        if dg < 0:
                            nc.scalar.activation(
                                esb[:], pssc[:],
                                mybir.ActivationFunctionType.Exp,
                                scale=SCALE, bias=bias_sb[:, 0:1],
                            )
                        else:
                            c0 = P * dg
                            nc.vector.tensor_add(
                                pssc[:, c0:c0 + P], pssc[:, c0:c0 + P],
                                tri_sb[:],
                            )
                            nc.scalar.activation(
                                esb[:, c0:TT2], pssc[:, c0:TT2],
                                mybir.ActivationFunctionType.Exp,
                                scale=SCALE, bias=bias_sb[:, 0:1],
                            )
                        hs.setdefault("avq", []).append((t_idx, st, esb))
                    return go

                def mk_av(hs=hs, b=b):
                    def go():
                        t_idx, st, esb = hs["avq"].pop(0)
                        n_s = 4 * (t_idx + 1)
                        dg = st - 4 * t_idx
                        z0 = P * dg if dg > 0 else 0
                        nc.tensor.matmul(
                            hs["psO"][:, z0:TT2], vS[b][:, st, :],
                            esb[:, z0:TT2],
                            start=(st == 0), stop=(st == n_s - 1),
                        )
                        hs.setdefault("denq", []).append((t_idx, st, esb, z0))
                    return go

                def mk_quad(hs=hs):
                    # 4 denominator matmuls, one per 32-wide PE column strip:
                    # issued back-to-back they run concurrently (own XBUS
                    # each), so the quad costs ~one matmul of stream time.
                    def go():
                        four = hs["denq"][:4]
                        del hs["denq"][:4]
                        for t_idx, st, esb, z0 in four:
                            g = st % 4
                            nc.tensor.matmul(
                                hs["psden"][32 * g:32 * g + 4, z0:TT2],
                                colC4_sb[:, 4 * t_idx:4 * t_idx + 4],
                                esb[:, z0:TT2],
                                start=(t_idx == 0 and st == g),
                                stop=(t_idx == NT2 - 1 and st == 12 + g),
                                tile_position=(0, 32 * g),
                            )
                    return go

                def mk_aoun(t_idx, hs=hs):
                    def go():
                        aoun = opool.tile([P, TT2], F16, tag="aoun",
                                          name="aoun", bufs=8)
                        nc.vector.tensor_scalar_mul(aoun[:], hs["psO"][:],
                                                    1.0 / 256.0)
                        hs["aouns"].append(aoun)
                    return go

                def mk_recip(hs=hs):
                    def go():
                        # combine the 4 strip-partials: PSUM->SBUF copy, then
                        # a [128,4]x[128,512] select-matmul sums the strips.
                        dsb = opool.tile([P, TT2], BF16, tag="dsb",
                                         name="dsb", bufs=2)
                        nc.vector.tensor_copy(dsb[:], hs["psden"][:])
                        pdf = ps_sc.tile([4, TT2], F32, tag="pssc",
                                         name="psdenf")
                        nc.tensor.matmul(pdf[:], sel4_sb[:], dsb[:],
                                         start=True, stop=True)
                        rc4 = opool.tile([4, TT2], F32, tag="recip4",
                                         name="recip4", bufs=1)
                        nc.vector.reciprocal_approx_fast(rc4[:], pdf[:])
                        rc4b = opool.tile([4, TT2], BF16, tag="rc4b",
                                          name="rc4b", bufs=1)
                        nc.vector.tensor_copy(rc4b[:], rc4[:])
                        rf = opool.tile([1, 4, TT2], BF16, tag="rflat",
                                        name="rflat", bufs=1)
                        nc.gpsimd.dma_start(rf[:], rc4b[:])
                        hs["rflat"] = rf
                    return go

                def mk_norm(t_idx, h=h, hs=hs, b=b):
                    def go():
                        psbc = ps_sc.tile([P, TT2], F32, tag="pssc",
                                          name="psbc")
                        nc.tensor.matmul(psbc[:], ones32[:],
                                         hs["rflat"][:, t_idx, :],
                                         start=True, stop=True)
                        nc.vector.tensor_mul(
                            aoT[b][:, h, t_idx * TT2:(t_idx + 1) * TT2],
                            hs["aouns"][t_idx][:], psbc[:],
                        )
                    return go

                flush_into(0)       # prev head's recip before psden realloc
                for t_idx in range(NT2):
                    n_s = 4 * (t_idx + 1)
                    nav = 0
                    for st in range(n_s):
                        if t_idx == 2 and st == 4:
                            flush_into(1)
                        elif t_idx == 3 and st == 4:
                            flush_into(2)
                        steps.append(mk_scores(t_idx, st))
                        if st >= 2:
                            steps.append(mk_av())
                            nav += 1
                            if nav % 4 == 0:
                                steps.append(mk_quad())
                    for _ in range(2):
                        steps.append(mk_av())
                        nav += 1
                        if nav % 4 == 0:
                            steps.append(mk_quad())
                    steps.append(mk_aoun(t_idx))

                pending.append((0, mk_recip()))
                pending.append((1, mk_norm(0)))
                pending.append((1, mk_norm(1)))
                pending.append((2, mk_norm(2)))
                pending.append((2, mk_norm(3)))

            for stage in (0, 1, 2):
                flush_into(stage)
            return steps

        def mk_pull(it, k):
            def pull():
                for _ in range(k):
                    fn = next(it, None)
                    if fn is None:
                        return
                    fn()
            return pull

        def drain(it):
            for fn in it:
                fn()

        # ================= Phase 3 emitter ================================
        def p3_ti(b, ti, pull, pswp, owpool, window, split_stores=False):
            lt0 = ti * TT3
            t0g = b * S + lt0
            wo_sb = wo_holder["wo"]
            for ii in range(8):
                psw = pswp.tile([P, IT3], F32, tag="psw", name="psw")
                for jo in range(QPC // P):
                    nc.tensor.matmul(
                        psw[:], aoT[b][:, jo, lt0:lt0 + TT3],
                        wo_sb[:, jo, ii * IT3:(ii + 1) * IT3],
                        start=(jo == 0), stop=(jo == QPC // P - 1),
                    )
                    if pull and jo == 1:
                        pull()
                ow = owpool.tile([P, IT3], F16, tag="ow", name="ow")
                if (window == "C" and ii % 4 == 3) or \
                        (split_stores and ii % 2 == 1):
                    nc.scalar.copy(ow[:], psw[:])
                else:
                    nc.vector.tensor_copy(ow[:], psw[:])
                if split_stores:
                    # final-drain tiles: halve each store and fan across all
                    # three queues so the kernel tail is shorter
                    e0 = (nc.sync, nc.gpsimd, nc.scalar)[(2 * ii) % 3]
                    e1 = (nc.sync, nc.gpsimd, nc.scalar)[(2 * ii + 1) % 3]
                    h = IT3 // 2
                    c0 = ii * IT3
                    e0.dma_start(out[t0g:t0g + TT3, c0:c0 + h], ow[:, 0:h])
                    e1.dma_start(out[t0g:t0g + TT3, c0 + h:c0 + IT3],
                                 ow[:, h:IT3])
                elif window == "C":
                    eng = nc.sync if ii % 2 == 0 else nc.gpsimd
                    eng.dma_start(out[t0g:t0g + TT3,
                                      ii * IT3:(ii + 1) * IT3], ow[:])
                else:
                    eng = (nc.sync, nc.gpsimd, nc.scalar)[ii % 3]
                    eng.dma_start(out[t0g:t0g + TT3,
                                      ii * IT3:(ii + 1) * IT3], ow[:])
                if pull:
                    pull()

        # ================= Window A: P1(b0) ===============================
        for ti in range(4):
            p1_ti(ti, None)

        # ================= Window B: P1(b1) + P2(b0) ======================
        gen0 = iter(build_p2_steps(0))
        pull0 = mk_pull(gen0, 2)
        for ti in range(4, 8):
            p1_ti(ti, pull0)
        for fn in p1pend:
            fn()
        p1pend.clear()
        drain(gen0)

        # ================= Window C pools =================================
        # emit a chunk of attention(b1) steps BEFORE the pool-release
        # barrier so the PE has work while window-B stragglers retire and
        # the wo chunk-0 DMA streams in.
        gen1 = iter(build_p2_steps(1))
        for _ in range(45):
            fn = next(gen1, None)
            if fn:
                fn()
        spool.release()
        xpool.release()
        wpool.release()
        p1acc.release()
        wopool = tc.alloc_tile_pool(name="wop", bufs=1)
        wo_sb = wopool.tile([P, QPC // P, DIM], F16)
        wo_holder["wo"] = wo_sb
        for c0, c1 in ((0, 512), (512, 1536), (1536, 2560), (2560, 4096)):
            nc.sync.dma_start(wo_sb[:, :, c0:c1], woR[:, :, c0:c1])
        aopool1 = tc.alloc_tile_pool(name="aop1", bufs=1)
        aoT[1] = aopool1.tile([P, NHEAD, S], F16, name="aoTb1")
        pswp = tc.alloc_tile_pool(name="psw", bufs=3, space="PSUM")
        owpool = tc.alloc_tile_pool(name="owp", bufs=6)

        # ================= Window C: P3(b0) + P2(b1) ======================
        pull1 = mk_pull(gen1, 2)
        for ti in range(S // TT3):
            p3_ti(0, ti, pull1, pswp, owpool, "C")
        drain(gen1)

        # ================= Window D pools =================================
        pswp.release()
        ps_den.release()
        ps_o.release()
        ps_sc.release()
        pswD = tc.alloc_tile_pool(name="pswD", bufs=6, space="PSUM")
        owD = tc.alloc_tile_pool(name="owD", bufs=8)

        # ================= Window D: P3(b1) ===============================
        for ti in range(S // TT3):
            p3_ti(1, ti, None, pswD, owD, "D",
                  split_stores=(ti >= S // TT3 - 2))

        # pop everything in LIFO order
        owD.release()
        pswD.release()
        owpool.release()
        pswp_released = True  # noqa: F841  (pswp already released above)
        aopool1.release()
        wopool.release()
        opool.release()
        epool.release()
        aopool0.release()
        kvpool.release()
        cpool.release()

    return nc


_NC_CACHE = {}


def _host_inputs(x, wq, wk, wv, wo, freqs_cos, freqs_sin):
    f16 = np.float16
    # pre-tiled x: xR[ti, g, p, k8, t] = x[ti*512+t, g*1024 + k8*128 + p]
    x2 = x.reshape(T, DIM).astype(f16)
    xRh = np.ascontiguousarray(
        x2.reshape(NT1, TT1, 4, 8, P).transpose(0, 2, 4, 3, 1))
    # de-interleave rope pairs: per head, rows [0,2,..,126, 1,3,..,127]
    perm = np.concatenate([np.arange(0, HD, 2), np.arange(1, HD, 2)])
    qidx = np.concatenate([h * HD + perm for h in range(NHEAD)])
    cosT = freqs_cos.T.astype(np.float32)          # (HD//2, S)
    sinT = freqs_sin.T.astype(np.float32)
    C = np.ascontiguousarray(np.concatenate([cosT, cosT], 0), dtype=f16)
    Sp = np.ascontiguousarray(np.concatenate([-sinT, sinT], 0), dtype=f16)
    colC4 = np.zeros((P, 4 * NT2), np.float32)
    for t in range(NT2):
        colC4[:, 4 * t + t] = 1.0 / 256.0
    colC4 = colC4.astype(ml_dtypes.bfloat16)
    sel4 = np.zeros((P, 4), np.float32)
    for g in range(4):
        for j in range(4):
            sel4[32 * g + j, j] = 1.0
    sel4 = sel4.astype(ml_dtypes.bfloat16)
    tri = np.where(np.arange(P)[None, :] >= np.arange(P)[:, None],
                   0.0, -1e9).astype(np.float32)
    biasv = np.full((P, 1), -4.0, np.float32)

    in_maps = []
    for c in range(N_CORES):
        wq_c = wq[c * QPC:(c + 1) * QPC, :][qidx, :]
        wk_c = wk[c * HD:(c + 1) * HD, :][perm, :]
        wqTh = wq_c.T.astype(f16)           # [DIM, QPC]
        wkTh = wk_c.T.astype(f16)           # [DIM, HD]
        wvTh = wv[c * HD:(c + 1) * HD, :].T.astype(f16)
        woTh = wo[:, c * QPC:(c + 1) * QPC].T.astype(f16)   # [QPC, DIM]
        in_maps.append({
            "xR": xRh,
            "wqR": np.ascontiguousarray(
                wqTh.reshape(4, 8, P, QPC).transpose(0, 2, 1, 3)),
            "wkR": np.ascontiguousarray(
                wkTh.reshape(KO, P, HD).transpose(1, 0, 2)),
            "wvR": np.ascontiguousarray(
                wvTh.reshape(KO, P, HD).transpose(1, 0, 2)),
            "woR": np.ascontiguousarray(
                woTh.reshape(QPC // P, P, DIM).transpose(1, 0, 2)),
            "ropeC": C,
            "ropeS": Sp,
            "colC4M": colC4,
            "sel4M": sel4,
            "triM": tri,
            "biasM": biasv,
        })
    return in_maps


def kernel(x, wq, wk, wv, wo, freqs_cos, freqs_sin, start_pos, _trace=False):
    x = np.asarray(x, np.float32)
    in_maps = _host_inputs(
        x, np.asarray(wq, np.float32), np.asarray(wk, np.float32),
        np.asarray(wv, np.float32), np.asarray(wo, np.float32),
        np.asarray(freqs_cos, np.float32), np.asarray(freqs_sin, np.float32),
    )
    if "nc" not in _NC_CACHE:
        nc = build_kernel()
        nc.compile()
        _NC_CACHE["nc"] = nc
    res = run_bass_kernel_spmd(
        _NC_CACHE["nc"], in_maps, list(range(N_CORES)), trace=_trace
    )
    acc = res.results[0]["out_part"].astype(np.float32)
    for c in range(1, N_CORES):
        acc += res.results[c]["out_part"].astype(np.float32)
    out = acc.reshape(B, S, DIM)
    if _trace:
        return out, res
    return out


# revision 33
# speedup vs baseline: 1.0170x; 1.0170x over previous
"""GQA attention (B=2,S=2048,DIM=4096,NH=32,NKV=8,HD=128) on 8 TRN2 NeuronCores.

Tensor-parallel over KV groups: core c owns q-heads [4c,4c+4), kv-head c and
wo columns [512c,512c+512). x replicated (feature-major fp16); each core emits
a partial (T,DIM) fp16 wo-output; host sums the 8 partials in fp32.

Fused-pipeline design (~788us vs 1012us for the 3-phase serial baseline):
- One instruction stream, overlapped windows: attention(b0) interleaves
  (pull-based, ~2 steps per 4 projection matmuls) into batch-1 QKV GEMMs,
  attention(b1) into batch-0 wo GEMMs.  The scalar-engine-bound softmax
  (~26us/head of Exp) hides behind peak-rate PE streams, and the PE runs at
  its 216ns/512-col issue rate through phase transitions.
- q/k/v and attention outputs live entirely in SBUF (no DRAM roundtrips).
- RoPE pair-swap via two contiguous partition-block DMA copies (wq/wk rows
  de-interleaved per head on host); V transpose on the DMA xbar engine;
  zero PE/PSUM cost for either.
- Softmax denominators: per 4 score tiles, 4 column-strip matmuls
  (tile_position=(0,32g)) issued back-to-back execute concurrently
  (~310ns per quad instead of 4 full streams), accumulated in one PSUM
  bank; strips summed by a [128,4] select-matmul, 1/den via
  reciprocal_approx_fast, broadcast over partitions by a bf16 rank-1
  matmul.
- All DRAM operands pre-tiled on host so every load is a contiguous
  >=8KB-per-partition burst; x streams as 1MB quarters over the three DMA
  issue queues (sync/scalar/gpsimd), double-buffered one ti ahead.
- P1 runs in 3-slice waves (kg-outer) matching first-chunk DMA arrival.
- PSUM: P1 3 accs + 3 score + 1 attn-out + 1 den (windows C/D retire the
  P1/P2 pools for 3/6 wo-accumulator banks).
"""

import math

import ml_dtypes
import numpy as np

import concourse.bass as bass
import concourse.mybir as mybir
import concourse.tile as tile
from concourse import bacc
from concourse.bass_utils import run_bass_kernel_spmd

B, S, DIM = 2, 2048, 4096
NH, NKV, HD = 32, 8, 128
T = B * S
N_CORES = 8
QPC = (NH // N_CORES) * HD          # 512 q-dims per core
NHEAD = NH // N_CORES               # 4 q heads per core
P = 128
F32 = mybir.dt.float32
F16 = mybir.dt.float16
BF16 = mybir.dt.bfloat16
SCALE = 1.0 / math.sqrt(HD)

TT1 = 512                           # phase-1 token tile
NT1 = T // TT1                      # 8
KO = DIM // P                       # 32 contraction tiles
TT2 = 512                           # phase-2 t tile
NT2 = S // TT2                      # 4 t-tiles per (b,h)
TT3 = 128                           # phase-3 token tile
IT3 = 512                           # phase-3 output-column tile


def build_kernel() -> bass.Bass:
    nc = bacc.Bacc()

    # all big operands arrive pre-tiled so every DMA is a contiguous
    # per-partition burst (8KB+) instead of 1KB strided descriptors
    xR = nc.declare_dram_parameter("xR", [NT1, 4, P, 8, TT1], F16,
                                   isOutput=False)
    wqR = nc.declare_dram_parameter("wqR", [4, P, 8, QPC], F16,
                                    isOutput=False)
    wkR = nc.declare_dram_parameter("wkR", [P, KO, HD], F16, isOutput=False)
    wvR = nc.declare_dram_parameter("wvR", [P, KO, HD], F16, isOutput=False)
    woR = nc.declare_dram_parameter("woR", [P, QPC // P, DIM], F16,
                                    isOutput=False)
    ropeC = nc.declare_dram_parameter("ropeC", [P, S], F16, isOutput=False)
    ropeS = nc.declare_dram_parameter("ropeS", [P, S], F16, isOutput=False)
    colC4M = nc.declare_dram_parameter("colC4M", [P, 4 * NT2], BF16,
                                       isOutput=False)
    sel4M = nc.declare_dram_parameter("sel4M", [P, 4], BF16, isOutput=False)
    triM = nc.declare_dram_parameter("triM", [P, P], F32, isOutput=False)
    biasM = nc.declare_dram_parameter("biasM", [P, 1], F32, isOutput=False)
    out = nc.declare_dram_parameter("out_part", [T, DIM], F16, isOutput=True)

    with tile.TileContext(nc) as tc:
        # ------------- pools; stack allocator => LIFO release discipline.
        cpool = tc.alloc_tile_pool(name="ct", bufs=1)
        kvpool = tc.alloc_tile_pool(name="qkv", bufs=1)
        aopool0 = tc.alloc_tile_pool(name="aop0", bufs=1)
        epool = tc.alloc_tile_pool(name="ep", bufs=7)
        opool = tc.alloc_tile_pool(name="op", bufs=2)
        ps_sc = tc.alloc_tile_pool(name="p2sc", bufs=3, space="PSUM")
        ps_o = tc.alloc_tile_pool(name="p2o", bufs=1, space="PSUM")
        ps_den = tc.alloc_tile_pool(name="p2d", bufs=1, space="PSUM")
        wpool = tc.alloc_tile_pool(name="w1", bufs=1)
        xpool = tc.alloc_tile_pool(name="xp", bufs=6)
        spool = tc.alloc_tile_pool(name="sp", bufs=2)
        p1acc = tc.alloc_tile_pool(name="p1a", bufs=3, space="PSUM")

        # ------------- persistent SBUF tensors
        qT = {b: kvpool.tile([P, NHEAD, S], F16, name=f"qTb{b}")
              for b in range(B)}
        kT = {b: kvpool.tile([P, S], F16, name=f"kTb{b}") for b in range(B)}
        vS = {b: kvpool.tile([P, S // P, P], BF16, name=f"vSb{b}")
              for b in range(B)}
        aoT = {0: aopool0.tile([P, NHEAD, S], F16, name="aoTb0")}
        wo_holder = {}

        # ------------- x streaming (4 quarters per ti, multi-queue)
        xmap = {}

        def ensure_x(ti):
            if ti >= NT1 or ti in xmap:
                return
            qs = []
            if ti == 0:
                # halves land ~2x sooner; subtile deps let ko0-3 start on h0
                half_eng = [(nc.scalar, nc.scalar), (nc.sync, nc.sync),
                            (nc.scalar, nc.gpsimd), (nc.sync, nc.gpsimd)]
                for g in range(4):
                    xq = xpool.tile([P, 8, TT1], F16, tag="xq", name="xq")
                    e0, e1 = half_eng[g]
                    e0.dma_start(xq[:, 0:4, :], xR[ti, g, :, 0:4, :])
                    e1.dma_start(xq[:, 4:8, :], xR[ti, g, :, 4:8, :])
                    qs.append(xq)
                xmap[ti] = qs
                return
            for g in range(4):
                if ti < 4:
                    eng = nc.scalar if g % 2 == 0 else nc.sync
                else:
                    eng = nc.sync
                xq = xpool.tile([P, 8, TT1], F16, tag="xq", name="xq")
                eng.dma_start(xq[:], xR[ti, g])
                qs.append(xq)
            xmap[ti] = qs

        # ------------- HAM warmup: the DMA path delivers nothing for the
        # first ~10us; keep the PE busy on a zeroed scratch tile so the
        # clock-gate opens (1.2->2.4GHz) before real operands land.
        junk = cpool.tile([P, TT1], F16)
        nc.vector.memset(junk[:], 0.0)
        jps = p1acc.tile([P, TT1], F32, tag="acc", name="jps")
        for _ in range(14):
            nc.tensor.matmul(jps[:], junk[:, 0:P], junk[:],
                             start=True, stop=True)

        # ------------- weight / table loads, interleaved with ti0's x so
        # every queue streams what the first sweeps need, in order.
        wq_sb = wpool.tile([P, KO, QPC], F16)
        nc.sync.dma_start(wq_sb[:, 0:4, :], wqR[0][:, 0:4, :])
        nc.sync.dma_start(wq_sb[:, 4:8, :], wqR[0][:, 4:8, :])
        nc.gpsimd.dma_start(wq_sb[:, 8:16, :], wqR[1])
        ensure_x(0)
        nc.scalar.dma_start(wq_sb[:, 16:24, :], wqR[2])
        nc.gpsimd.dma_start(wq_sb[:, 24:32, :], wqR[3])
        wk_sb = wpool.tile([P, KO, HD], F16)
        nc.gpsimd.dma_start(wk_sb[:], wkR[:])
        wv_sb = wpool.tile([P, KO, HD], F16)
        nc.gpsimd.dma_start(wv_sb[:], wvR[:])
        ropeC_sb = wpool.tile([P, S], F16)
        nc.sync.dma_start(ropeC_sb[:], ropeC[:])
        ropeS_sb = wpool.tile([P, S], F16)
        nc.sync.dma_start(ropeS_sb[:], ropeS[:])
        colC4_sb = cpool.tile([P, 4 * NT2], BF16)
        nc.sync.dma_start(colC4_sb[:], colC4M[:])
        sel4_sb = cpool.tile([P, 4], BF16)
        nc.sync.dma_start(sel4_sb[:], sel4M[:])
        tri_sb = cpool.tile([P, P], F32)
        nc.sync.dma_start(tri_sb[:], triM[:])
        bias_sb = cpool.tile([P, 1], F32)
        nc.sync.dma_start(bias_sb[:], biasM[:])
        ones32 = cpool.tile([1, P], BF16)
        nc.gpsimd.memset(ones32[:], 1.0)

        # ================= Phase 1 machinery ==============================
        def rope_tail(b, lt0, j, acc):
            def go():
                raw = spool.tile([P, TT1], F16, tag="raw", name="raw")
                nc.any.tensor_copy(raw[:], acc[:])
                # pair-swap == swap of the (deinterleaved) top/bottom halves
                swp = spool.tile([P, TT1], F16, tag="swp", name="swp")
                nc.gpsimd.dma_start(swp[0:64, :], raw[64:128, :])
                nc.gpsimd.dma_start(swp[64:128, :], raw[0:64, :])
                rc = spool.tile([P, TT1], F16, tag="rc", name="rc")
                nc.vector.tensor_mul(rc[:], raw[:], ropeC_sb[:, lt0:lt0 + TT1])
                rs = spool.tile([P, TT1], F16, tag="rs", name="rs")
                nc.vector.tensor_mul(rs[:], swp[:], ropeS_sb[:, lt0:lt0 + TT1])
                dst = (qT[b][:, j, lt0:lt0 + TT1] if j < 4
                       else kT[b][:, lt0:lt0 + TT1])
                nc.vector.tensor_add(dst, rc[:], rs[:])
            return go

        def v_tail(b, lt0, acc):
            def go():
                vraw = spool.tile([P, TT1], BF16, tag="vraw", name="vraw")
                nc.any.tensor_copy(vraw[:], acc[:])
                so0 = lt0 // P
                for jj in range(TT1 // P):
                    nc.sync.dma_start_transpose(
                        vS[b][:, so0 + jj, :], vraw[:, jj * P:(jj + 1) * P])
            return go

        def w_of(a):
            if a < 4:
                return wq_sb, a * P
            return (wk_sb, 0) if a == 4 else (wv_sb, 0)

        p1pend = []

        def p1_ti(ti, pull):
            b = ti // (NT1 // B)
            lt0 = (ti % (NT1 // B)) * TT1
            ensure_x(ti)
            qs = xmap[ti]
            waves = [(0, 1, 2), (3, 4, 5)]
            for wi, sl in enumerate(waves):
                # previous wave's tails must retire before accs rotate
                for fn in p1pend:
                    fn()
                p1pend.clear()
                if pull:
                    pull()
                accs = [p1acc.tile([P, TT1], F32, tag="acc", name="acc")
                        for _ in sl]
                for kg in range(4):
                    for ai, a in enumerate(sl):
                        w_sb, c0 = w_of(a)
                        for k8 in range(8):
                            ko = kg * 8 + k8
                            nc.tensor.matmul(
                                accs[ai][:], w_sb[:, ko, c0:c0 + P],
                                qs[kg][:, k8, :],
                                start=(ko == 0), stop=(ko == KO - 1),
                            )
                            if pull and k8 == 3:
                                pull()
                        if pull:
                            pull()
                if wi == 0:
                    ensure_x(ti + 1)
                for ai, a in enumerate(sl):
                    if a < 5:
                        p1pend.append(rope_tail(b, lt0, a, accs[ai]))
                    else:
                        p1pend.append(v_tail(b, lt0, accs[ai]))

        # ================= Phase 2 step list ==============================
        def build_p2_steps(b):
            steps = []
            pending = []            # (stage, closure) deferred to next head

            def flush_into(stage):
                keep = []
                for stg, fn in pending:
                    if stg == stage:
                        steps.append(fn)
                    else:
                        keep.append((stg, fn))
                pending[:] = keep

            for h in range(NHEAD):
                hs = {}

                def mk_scores(t_idx, st, h=h, hs=hs, b=b):
                    def go():
                        if t_idx == 0 and st == 0:
                            hs["psden"] = ps_den.tile([P, TT2], F32,
                                                      tag="psden", name="psden")
                            hs["aouns"] = []
                        if st == 0:
                            hs["psO"] = ps_o.tile([P, TT2], F32, tag="psO",
                                                  name="psO")
                        t0 = t_idx * TT2
                        pssc = ps_sc.tile([P, TT2], F32, tag="pssc",
                                          name="pssc")
                        nc.tensor.matmul(
                            pssc[:], kT[b][:, st * P:(st + 1) * P],
                            qT[b][:, h, t0:t0 + TT2], start=True, stop=True,
                        )
                        dg = st - 4 * t_idx
                        esb = epool.tile([P, TT2], BF16, tag="esb", name="esb")
                        if dg < 0:
                            nc.scalar.activation(
                                esb[:], pssc[:],
                                mybir.ActivationFunctionType.Exp,
                                scale=SCALE, bias=bias_sb[:, 0:1],
                            )
                        else:
                            c0 = P * dg
                            nc.vector.tensor_add(
                                pssc[:, c0:c0 + P], pssc[:, c0:c0 + P],
                                tri_sb[:],
                            )
                            nc.scalar.activation(
                                esb[:, c0:TT2], pssc[:, c0:TT2],
                                mybir.ActivationFunctionType.Exp,
                                scale=SCALE, bias=bias_sb[:, 0:1],
                            )
                        hs.setdefault("avq", []).append((t_idx, st, esb))
                    return go

                def mk_av(hs=hs, b=b):
                    def go():
                        t_idx, st, esb = hs["avq"].pop(0)
                        n_s = 4 * (t_idx + 1)
                        dg = st - 4 * t_idx
                        z0 = P * dg if dg > 0 else 0
                        nc.tensor.matmul(
                            hs["psO"][:, z0:TT2], vS[b][:, st, :],
                            esb[:, z0:TT2],
                            start=(st == 0), stop=(st == n_s - 1),
                        )
                        hs.setdefault("denq", []).append((t_idx, st, esb, z0))
                    return go

                def mk_quad(hs=hs):
                    # 4 denominator matmuls, one per 32-wide PE column strip:
                    # issued back-to-back they run concurrently (own XBUS
                    # each), so the quad costs ~one matmul of stream time.
                    def go():
                        four = hs["denq"][:4]
                        del hs["denq"][:4]
                        for t_idx, st, esb, z0 in four:
                            g = st % 4
                            nc.tensor.matmul(
                                hs["psden"][32 * g:32 * g + 4, z0:TT2],
                                colC4_sb[:, 4 * t_idx:4 * t_idx + 4],
                                esb[:, z0:TT2],
                                start=(t_idx == 0 and st == g),
                                stop=(t_idx == NT2 - 1 and st == 12 + g),
                                tile_position=(0, 32 * g),
                            )
                    return go

                def mk_aoun(t_idx, hs=hs):
                    def go():
                        aoun = opool.tile([P, TT2], F16, tag="aoun",
                                          name="aoun", bufs=8)
                        nc.vector.tensor_scalar_mul(aoun[:], hs["psO"][:],
                                                    1.0 / 256.0)
                        hs["aouns"].append(aoun)
                    return go

                def mk_recip(hs=hs):
                    def go():
                        # combine the 4 strip-partials: PSUM->SBUF copy, then
                        # a [128,4]x[128,512] select-matmul sums the strips.
                        dsb = opool.tile([P, TT2], BF16, tag="dsb",
                                         name="dsb", bufs=2)
                        nc.vector.tensor_copy(dsb[:], hs["psden"][:])
                        pdf = ps_sc.tile([4, TT2], F32, tag="pssc",
                                         name="psdenf")
                        nc.tensor.matmul(pdf[:], sel4_sb[:], dsb[:],
                                         start=True, stop=True)
                        rc4 = opool.tile([4, TT2], F32, tag="recip4",
                                         name="recip4", bufs=1)
                        nc.vector.reciprocal_approx_fast(rc4[:], pdf[:])
                        rc4b = opool.tile([4, TT2], BF16, tag="rc4b",
                                          name="rc4b", bufs=1)
                        nc.vector.tensor_copy(rc4b[:], rc4[:])
                        rf = opool.tile([1, 4, TT2], BF16, tag="rflat",
                                        name="rflat", bufs=1)
                        nc.gpsimd.dma_start(rf[:], rc4b[:])
                        hs["rflat"] = rf
                    return go

                def mk_norm(t_idx, h=h, hs=hs, b=b):
                    def go():
                        psbc = ps_sc.tile([P, TT2], F32, tag="pssc",
                                          name="psbc")
                        nc.tensor.matmul(psbc[:], ones32[:],
                                         hs["rflat"][:, t_idx, :],
                                         start=True, stop=True)
                        nc.vector.tensor_mul(
                            aoT[b][:, h, t_idx * TT2:(t_idx + 1) * TT2],
                            hs["aouns"][t_idx][:], psbc[:],
                        )
                    return go

                flush_into(0)       # prev head's recip before psden realloc
                for t_idx in range(NT2):
                    n_s = 4 * (t_idx + 1)
                    nav = 0
                    for st in range(n_s):
                        if t_idx == 2 and st == 4:
                            flush_into(1)
                        elif t_idx == 3 and st == 4:
                            flush_into(2)
                        steps.append(mk_scores(t_idx, st))
                        if st >= 2:
                            steps.append(mk_av())
                            nav += 1
                            if nav % 4 == 0:
                                steps.append(mk_quad())
                    for _ in range(2):
                        steps.append(mk_av())
                        nav += 1
                        if nav % 4 == 0:
                            steps.append(mk_quad())
                    steps.append(mk_aoun(t_idx))

                pending.append((0, mk_recip()))
                pending.append((1, mk_norm(0)))
                pending.append((1, mk_norm(1)))
                pending.append((2, mk_norm(2)))
                pending.append((2, mk_norm(3)))

            for stage in (0, 1, 2):
                flush_into(stage)
            return steps

        def mk_pull(it, k):
            def pull():
                for _ in range(k):
                    fn = next(it, None)
                    if fn is None:
                        return
                    fn()
            return pull

        def drain(it):
            for fn in it:
                fn()

        # ================= Phase 3 emitter ================================
        def p3_ti(b, ti, pull, pswp, owpool, window, split_stores=False):
            lt0 = ti * TT3
            t0g = b * S + lt0
            wo_sb = wo_holder["wo"]
            for ii in range(8):
                psw = pswp.tile([P, IT3], F32, tag="psw", name="psw")
                for jo in range(QPC // P):
                    nc.tensor.matmul(
                        psw[:], aoT[b][:, jo, lt0:lt0 + TT3],
                        wo_sb[:, jo, ii * IT3:(ii + 1) * IT3],
                        start=(jo == 0), stop=(jo == QPC // P - 1),
                    )
                    if pull and jo == 1:
                        pull()
                ow = owpool.tile([P, IT3], F16, tag="ow", name="ow")
                if (window == "C" and ii % 4 == 3) or \
                        (split_stores and ii % 2 == 1):
                    nc.scalar.copy(ow[:], psw[:])
                else:
                    nc.vector.tensor_copy(ow[:], psw[:])
                if split_stores:
                    # final-drain tiles: halve each store and fan across all
                    # three queues so the kernel tail is shorter
                    e0 = (nc.sync, nc.gpsimd, nc.scalar)[(2 * ii) % 3]
                    e1 = (nc.sync, nc.gpsimd, nc.scalar)[(2 * ii + 1) % 3]
                    h = IT3 // 2
                    c0 = ii * IT3
                    e0.dma_start(out[t0g:t0g + TT3, c0:c0 + h], ow[:, 0:h])
                    e1.dma_start(out[t0g:t0g + TT3, c0 + h:c0 + IT3],
                                 ow[:, h:IT3])
                elif window == "C":
                    eng = nc.sync if ii % 2 == 0 else nc.gpsimd
                    eng.dma_start(out[t0g:t0g + TT3,
                                      ii * IT3:(ii + 1) * IT3], ow[:])
                else:
                    eng = (nc.sync, nc.gpsimd, nc.scalar)[ii % 3]
                    eng.dma_start(out[t0g:t0g + TT3,
                                      ii * IT3:(ii + 1) * IT3], ow[:])
                if pull:
                    pull()

        # ================= Window A: P1(b0) ===============================
        for ti in range(4):
            p1_ti(ti, None)

        # ================= Window B: P1(b1) + P2(b0) ======================
        gen0 = iter(build_p2_steps(0))
        pull0 = mk_pull(gen0, 2)
        for ti in range(4, 8):
            p1_ti(ti, pull0)
        for fn in p1pend:
            fn()
        p1pend.clear()
        drain(gen0)

        # ================= Window C pools =================================
        # emit a chunk of attention(b1) steps BEFORE the pool-release
        # barrier so the PE has work while window-B stragglers retire and
        # the wo chunk-0 DMA streams in.
        gen1 = iter(build_p2_steps(1))
        for _ in range(45):
            fn = next(gen1, None)
            if fn:
                fn()
        spool.release()
        xpool.release()
        wpool.release()
        p1acc.release()
        wopool = tc.alloc_tile_pool(name="wop", bufs=1)
        wo_sb = wopool.tile([P, QPC // P, DIM], F16)
        wo_holder["wo"] = wo_sb
        for c0, c1 in ((0, 512), (512, 1536), (1536, 2560), (2560, 4096)):
            nc.sync.dma_start(wo_sb[:, :, c0:c1], woR[:, :, c0:c1])
        aopool1 = tc.alloc_tile_pool(name="aop1", bufs=1)
        aoT[1] = aopool1.tile([P, NHEAD, S], F16, name="aoTb1")
        pswp = tc.alloc_tile_pool(name="psw", bufs=3, space="PSUM")
        owpool = tc.alloc_tile_pool(name="owp", bufs=6)

        # ================= Window C: P3(b0) + P2(b1) ======================
        pull1 = mk_pull(gen1, 2)
        for ti in range(S // TT3):
            p3_ti(0, ti, pull1, pswp, owpool, "C")
        drain(gen1)

        # ================= Window D pools =================================
        pswp.release()
        ps_den.release()
        ps_o.release()
        ps_sc.release()
        pswD = tc.alloc_tile_pool(name="pswD", bufs=6, space="PSUM")
        owD = tc.alloc_tile_pool(name="owD", bufs=8)

        # ================= Window D: P3(b1) ===============================
        for ti in range(S // TT3):
            p3_ti(1, ti, None, pswD, owD, "D",
                  split_stores=(ti >= S // TT3 - 2))

        # pop everything in LIFO order
        owD.release()
        pswD.release()
        owpool.release()
        pswp_released = True  # noqa: F841  (pswp already released above)
        aopool1.release()
        wopool.release()
        opool.release()
        epool.release()
        aopool0.release()
        kvpool.release()
        cpool.release()

    return nc


_NC_CACHE = {}


def _host_inputs(x, wq, wk, wv, wo, freqs_cos, freqs_sin):
    f16 = np.float16
    # pre-tiled x: xR[ti, g, p, k8, t] = x[ti*512+t, g*1024 + k8*128 + p]
    x2 = x.reshape(T, DIM).astype(f16)
    xRh = np.ascontiguousarray(
        x2.reshape(NT1, TT1, 4, 8, P).transpose(0, 2, 4, 3, 1))
    # de-interleave rope pairs: per head, rows [0,2,..,126, 1,3,..,127]
    perm = np.concatenate([np.arange(0, HD, 2), np.arange(1, HD, 2)])
    qidx = np.concatenate([h * HD + perm for h in range(NHEAD)])
    cosT = freqs_cos.T.astype(np.float32)          # (HD//2, S)
    sinT = freqs_sin.T.astype(np.float32)
    C = np.ascontiguousarray(np.concatenate([cosT, cosT], 0), dtype=f16)
    Sp = np.ascontiguousarray(np.concatenate([-sinT, sinT], 0), dtype=f16)
    colC4 = np.zeros((P, 4 * NT2), np.float32)
    for t in range(NT2):
        colC4[:, 4 * t + t] = 1.0 / 256.0
    colC4 = colC4.astype(ml_dtypes.bfloat16)
    sel4 = np.zeros((P, 4), np.float32)
    for g in range(4):
        for j in range(4):
            sel4[32 * g + j, j] = 1.0
    sel4 = sel4.astype(ml_dtypes.bfloat16)
    tri = np.where(np.arange(P)[None, :] >= np.arange(P)[:, None],
                   0.0, -1e9).astype(np.float32)
    biasv = np.full((P, 1), -4.0, np.float32)

    in_maps = []
    for c in range(N_CORES):
        wq_c = wq[c * QPC:(c + 1) * QPC, :][qidx, :]
        wk_c = wk[c * HD:(c + 1) * HD, :][perm, :]
        wqTh = wq_c.T.astype(f16)           # [DIM, QPC]
        wkTh = wk_c.T.astype(f16)           # [DIM, HD]
        wvTh = wv[c * HD:(c + 1) * HD, :].T.astype(f16)
        woTh = wo[:, c * QPC:(c + 1) * QPC].T.astype(f16)   # [QPC, DIM]
        in_maps.append({
            "xR": xRh,
            "wqR": np.ascontiguousarray(
                wqTh.reshape(4, 8, P, QPC).transpose(0, 2, 1, 3)),
            "wkR": np.ascontiguousarray(
                wkTh.reshape(KO, P, HD).transpose(1, 0, 2)),
            "wvR": np.ascontiguousarray(
                wvTh.reshape(KO, P, HD).transpose(1, 0, 2)),
            "woR": np.ascontiguousarray(
                woTh.reshape(QPC // P, P, DIM).transpose(1, 0, 2)),
            "ropeC": C,
            "ropeS": Sp,
            "colC4M": colC4,
            "sel4M": sel4,
            "triM": tri,
            "biasM": biasv,
        })
    return in_maps


def kernel(x, wq, wk, wv, wo, freqs_cos, freqs_sin, start_pos, _trace=False):
    x = np.asarray(x, np.float32)
    in_maps = _host_inputs(
        x, np.asarray(wq, np.float32), np.asarray(wk, np.float32),
        np.asarray(wv, np.float32), np.asarray(wo, np.float32),
        np.asarray(freqs_cos, np.float32), np.asarray(freqs_sin, np.float32),
    )
    if "nc" not in _NC_CACHE:
        nc = build_kernel()
        nc.compile()
        _NC_CACHE["nc"] = nc
    res = run_bass_kernel_spmd(
        _NC_CACHE["nc"], in_maps, list(range(N_CORES)), trace=_trace
    )
    acc = res.results[0]["out_part"].astype(np.float32)
    for c in range(1, N_CORES):
        acc += res.results[c]["out_part"].astype(np.float32)
    out = acc.reshape(B, S, DIM)
    if _trace:
        return out, res
    return out
